# revision 1
# baseline (speedup 1.0000x reference)
"""Expert-parallel MoE SwiGLU kernel for one TRN2 chip (8 NeuronCores).

Problem: out[n] = sum_k w[n,k] * FFN_{idx[n,k]}(x[n]) with E=8 experts,
top-2 routing, H=1024, I=4096, N=2048 tokens.

Strategy: one expert per core. Tokens are routed (gathered) per expert on
the host, each core runs the three bf16 matmuls of its expert's SwiGLU FFN
(silu(x@w1) * (x@w3)) @ w2 over its token batch entirely transposed
(tokens along the PE moving/free dim), and the host scatter-adds the
returned per-expert outputs with the routing weights. Expert token counts
above the per-core capacity (PE moving-dim chunk of 512) spill to a small
host-side f32 pass so the device runs a single full-width chunk.
"""

import sys

for _p in ("/opt/trn_rl_repo", "/opt/pypackages"):
    if _p not in sys.path:
        sys.path.insert(0, _p)

import numpy as np
import ml_dtypes

import concourse.tile as tile
from concourse import bacc, mybir
from concourse.bass_utils import run_bass_kernel_spmd

P = 128
H = 1024
I = 4096
KH = H // P    # 8 contraction subtiles for the first matmuls
II = I // P    # 32 intermediate subtiles / contraction subtiles for w2
CAP = 512      # per-core token capacity (single PE moving chunk)
N_WARM = 22    # PE warmup matmuls (HAM clock ramp) during the input DMA wait

BF16 = mybir.dt.bfloat16
F32 = mybir.dt.float32


def _build(C):
    """One-expert SwiGLU FFN over C tokens (C <= 512), transposed layout.

    DRAM inputs (per core):
      xg   [P, KH, C]       bf16  x^T: [hp, kh, c] = x[tok c, kh*P+hp]
      w13t [II, P, 2, KH, P] bf16 [ii, hp, 0, kh, m] = w1[kh*P+hp, ii*P+m]
                                  [ii, hp, 1, kh, m] = w3[kh*P+hp, ii*P+m]
      w2t  [KH, P, II, P]   bf16  [hh, ip, ik, m] = w2[ik*P+ip, hh*P+m]
    Output:
      yt   [KH, P, C]       f32   y^T tiled by output subtile
    """
    assert C <= 512
    nc = bacc.Bacc("TRN2", target_bir_lowering=False, debug=False)
    xg = nc.dram_tensor("xg", [P, KH, C], BF16, kind="ExternalInput")
    w13t = nc.dram_tensor("w13t", [II, P, 2, KH, P], BF16, kind="ExternalInput")
    w2t = nc.dram_tensor("w2t", [KH, P, II, P], BF16, kind="ExternalInput")
    yt = nc.dram_tensor("yt", [KH, P, C], F32, kind="ExternalOutput")

    with tile.TileContext(nc) as tc:
        with (
            tc.tile_pool(name="xp", bufs=1) as xp,
            tc.tile_pool(name="pp", bufs=1) as pp,
            tc.tile_pool(name="wp", bufs=10) as wp,
            tc.tile_pool(name="w2p", bufs=2) as w2p,
            tc.tile_pool(name="gp", bufs=4) as gp,
            tc.tile_pool(name="yp", bufs=3) as yp,
            tc.tile_pool(name="warm", bufs=1) as warm,
            tc.tile_pool(name="psA", bufs=2, space="PSUM") as psA,
            tc.tile_pool(name="psB", bufs=2, space="PSUM") as psB,
            tc.tile_pool(name="psW", bufs=1, space="PSUM") as psW,
        ):
            # PE warmup: ramp the tensor engine to high-activity clock while
            # the input DMAs are in flight. Reads a zeroed tile, result is
            # never consumed.
            wtile = warm.tile([P, 512], BF16)
            nc.vector.memset(wtile[:], 0.0)
            wps = psW.tile([P, 512], F32)
            for i in range(N_WARM):
                nc.tensor.matmul(
                    wps, wtile[:, :P], wtile[:], start=(i == 0),
                    stop=(i == N_WARM - 1),
                )

            # Startup-critical loads: first weight tile, then x^T (8KB
            # contiguous per partition), then the second weight tile on the
            # other issue queue.
            w13sb0 = wp.tile([P, 2, KH, P], BF16, tag="w13")
            nc.sync.dma_start(w13sb0[:], w13t[0])
            xsb = xp.tile([P, KH, C], BF16)
            nc.sync.dma_start(xsb[:], xg[:])
            w13sb1 = wp.tile([P, 2, KH, P], BF16, tag="w13")
            nc.gpsimd.dma_start(w13sb1[:], w13t[1])

            psb = pp.tile([P, II, C], BF16)

            # Phase A: h1 = silu(x@w1), h3 = x@w3, p = h1*h3 (all transposed)
            for ii in range(II):
                if ii == 0:
                    w13sb = w13sb0
                elif ii == 1:
                    w13sb = w13sb1
                else:
                    w13sb = wp.tile([P, 2, KH, P], BF16, tag="w13")
                    nc.gpsimd.dma_start(w13sb[:], w13t[ii])
                pg = psA.tile([P, C], F32, tag="pg")
                pu = psA.tile([P, C], F32, tag="pu")
                for kh in range(KH):
                    nc.tensor.matmul(
                        pg,
                        w13sb[:, 0, kh, :],
                        xsb[:, kh, :],
                        start=(kh == 0),
                        stop=(kh == KH - 1),
                    )
                for kh in range(KH):
                    nc.tensor.matmul(
                        pu,
                        w13sb[:, 1, kh, :],
                        xsb[:, kh, :],
                        start=(kh == 0),
                        stop=(kh == KH - 1),
                    )
                gs = gp.tile([P, C], BF16, tag="g")
                nc.scalar.activation(gs, pg, mybir.ActivationFunctionType.Silu)
                nc.vector.tensor_tensor(
                    psb[:, ii, :], gs, pu, mybir.AluOpType.mult
                )

            # Phase B: y = p @ w2 (transposed: yT = w2T-contraction over I).
            # The last hh is split column-wise so its first half's copy+DMA
            # overlaps the second half's matmuls (shorter kernel tail).
            for hh in range(KH):
                w2sb = w2p.tile([P, II, P], BF16, tag="w2")
                nc.scalar.dma_start(w2sb[:], w2t[hh])
                halves = [(0, C)] if hh < KH - 1 else [
                    (0, C // 2), (C // 2, C - C // 2),
                ]
                for c0, cc in halves:
                    py = psB.tile([P, cc], F32, tag="py")
                    for ik in range(II):
                        nc.tensor.matmul(
                            py,
                            w2sb[:, ik, :],
                            psb[:, ik, c0 : c0 + cc],
                            start=(ik == 0),
                            stop=(ik == II - 1),
                        )
                    ysb = yp.tile([P, cc], F32, tag="y")
                    nc.scalar.copy(ysb, py)
                    nc.sync.dma_start(yt[hh, :, c0 : c0 + cc], ysb[:])

    nc.compile()
    return nc


_PROGRAM_CACHE = {}


def _host_swiglu(x, w1e, w2e, w3e):
    g = x @ w1e
    u = x @ w3e
    g = g / (1.0 + np.exp(-g))
    return (g * u) @ w2e


def kernel(x, expert_indices, expert_weights, w1, w2, w3):
    x = np.asarray(x, dtype=np.float32)
    idx = np.asarray(expert_indices)
    wts = np.asarray(expert_weights, dtype=np.float32)
    w1 = np.asarray(w1, dtype=np.float32)
    w2 = np.asarray(w2, dtype=np.float32)
    w3 = np.asarray(w3, dtype=np.float32)
    N = x.shape[0]
    E = w1.shape[0]
    bf16 = ml_dtypes.bfloat16

    # host-side routing: token list (with multiplicity) per expert; tokens
    # beyond CAP spill to the host f32 path (tiny tail, keeps device at one
    # full-width PE chunk)
    toks, tokw, spill_toks, spill_w = [], [], [], []
    for e in range(E):
        rows, cols = np.nonzero(idx == e)
        w_e = wts[rows, cols]
        toks.append(rows[:CAP])
        tokw.append(w_e[:CAP])
        spill_toks.append(rows[CAP:])
        spill_w.append(w_e[CAP:])
    C = max(16, max(len(t) for t in toks))
    C = ((C + 15) // 16) * 16

    if C not in _PROGRAM_CACHE:
        _PROGRAM_CACHE[C] = _build(C)
    nc = _PROGRAM_CACHE[C]

    in_maps = []
    for e in range(E):
        xt = np.zeros((C, H), dtype=np.float32)
        if len(toks[e]):
            xt[: len(toks[e])] = x[toks[e]]
        # [C, H] -> [hp, kh, c]
        xge = xt.T.reshape(KH, P, C).transpose(1, 0, 2)
        # w1/w3 [H, I] -> [ii, hp, {w1,w3}, kh, m]
        w13 = np.stack(
            [
                w1[e].reshape(KH, P, II, P).transpose(2, 1, 0, 3),
                w3[e].reshape(KH, P, II, P).transpose(2, 1, 0, 3),
            ],
            axis=2,
        )  # [II, P, 2, KH, P]
        in_maps.append(
            {
                "xg": np.ascontiguousarray(xge.astype(bf16)),
                "w13t": np.ascontiguousarray(w13.astype(bf16)),
                "w2t": np.ascontiguousarray(
                    w2[e].reshape(II, P, KH, P).transpose(2, 1, 0, 3).astype(bf16)
                ),
            }
        )

    res = run_bass_kernel_spmd(nc, in_maps, core_ids=list(range(E)))

    out = np.zeros((N, H), dtype=np.float32)
    for e in range(E):
        cnt = len(toks[e])
        if cnt:
            y = res.results[e]["yt"].reshape(H, C).T[:cnt]
            np.add.at(out, toks[e], y * tokw[e][:, None])
        if len(spill_toks[e]):
            ys = _host_swiglu(x[spill_toks[e]], w1[e], w2[e], w3[e])
            np.add.at(out, spill_toks[e], ys * spill_w[e][:, None])
    return out



# revision 5
# speedup vs baseline: 1.1512x; 1.1512x over previous
"""Expert-parallel MoE SwiGLU kernel for one TRN2 chip (8 NeuronCores).

Problem: out[n] = sum_k w[n,k] * FFN_{idx[n,k]}(x[n]) with E=8 experts,
top-2 routing, H=1024, I=4096, N=2048 tokens.

Strategy: one expert per core. Tokens are routed (gathered) per expert on
the host, each core runs the three bf16 matmuls of its expert's SwiGLU FFN
(silu(x@w1) * (x@w3)) @ w2 over its token batch entirely transposed
(tokens along the PE moving/free dim), and the host scatter-adds the
returned per-expert outputs with the routing weights. Expert token counts
above the per-core capacity (PE moving-dim chunk of 512) spill to a small
host-side f32 pass so the device runs a single full-width chunk.

Schedule notes (from NTFF profile analysis): the kernel is tensor-bound
(768 N=512 bf16 matmuls ~= 166us at 2.4GHz). The startup-critical inputs
(x 1MB + first w13 tile 0.5MB) are split across the three independent DMA
rings (sync + scalar hardware DGE, gpsimd software DGE, ~150GB/s each) so
the first real matmul issues ~11us after kernel start instead of ~21us.
The w13/w2 weight streams alternate between the gpsimd and scalar rings so
each ring only needs ~75GB/s sustained. PE warmup matmuls (HAM clock ramp)
cover the input DMA window.
"""

import sys

for _p in ("/opt/trn_rl_repo", "/opt/pypackages"):
    if _p not in sys.path:
        sys.path.insert(0, _p)

import numpy as np
import ml_dtypes

import concourse.tile as tile
from concourse import bacc, mybir
from concourse.bass_utils import run_bass_kernel_spmd

P = 128
H = 1024
I = 4096
KH = H // P    # 8 contraction subtiles for the first matmuls
KH2 = KH // 2  # kh subtiles per x half-tile
II = I // P    # 32 intermediate subtiles / contraction subtiles for w2
CAP = 512      # per-core token capacity (single PE moving chunk)
N_WARM = 20    # PE warmup matmuls (HAM clock ramp) during the input DMA wait
W_COLS = 256   # warmup matmul moving width (finer tail granularity)

BF16 = mybir.dt.bfloat16
F32 = mybir.dt.float32


def _build(C):
    """One-expert SwiGLU FFN over C tokens (C <= 512), transposed layout.

    DRAM inputs (per core):
      xg   [P, KH, C]       bf16  x^T: [hp, kh, c] = x[tok c, kh*P+hp]
      w13t [II, P, 2, KH, P] bf16 [ii, hp, 0, kh, m] = w1[kh*P+hp, ii*P+m]
                                  [ii, hp, 1, kh, m] = w3[kh*P+hp, ii*P+m]
      w2t  [KH, P, II, P]   bf16  [hh, ip, ik, m] = w2[ik*P+ip, hh*P+m]
    Output:
      yt   [KH, P, C]       f32   y^T tiled by output subtile
    """
    assert C <= 512
    nc = bacc.Bacc("TRN2", target_bir_lowering=False, debug=False)
    xg = nc.dram_tensor("xg", [P, KH, C], BF16, kind="ExternalInput")
    w13t = nc.dram_tensor("w13t", [II, P, 2, KH, P], BF16, kind="ExternalInput")
    w2t = nc.dram_tensor("w2t", [KH, P, II, P], BF16, kind="ExternalInput")
    yt = nc.dram_tensor("yt", [KH, P, C], F32, kind="ExternalOutput")

    with tile.TileContext(nc) as tc:
        with (
            tc.tile_pool(name="xp", bufs=1) as xp,
            tc.tile_pool(name="pp", bufs=1) as pp,
            tc.tile_pool(name="wp", bufs=10) as wp,
            tc.tile_pool(name="w2p", bufs=2) as w2p,
            tc.tile_pool(name="gp", bufs=4) as gp,
            tc.tile_pool(name="yp", bufs=3) as yp,
            tc.tile_pool(name="warm", bufs=1) as warm,
            tc.tile_pool(name="psA", bufs=3, space="PSUM") as psA,
            tc.tile_pool(name="psB", bufs=2, space="PSUM") as psB,
        ):
            # Startup-critical loads, one per independent DMA ring so they
            # move in parallel (~150GB/s each): x halves on the two hardware
            # DGE rings (sync + scalar), first two w13 tiles on the gpsimd
            # software DGE ring.
            xa = xp.tile([P, KH2, C], BF16)
            nc.sync.dma_start(xa[:], xg[:, 0:KH2, :])
            xb = xp.tile([P, KH2, C], BF16)
            nc.scalar.dma_start(xb[:], xg[:, KH2:KH, :])
            w13sb0 = wp.tile([P, 2, KH, P], BF16, tag="w13")
            nc.gpsimd.dma_start(w13sb0[:], w13t[0])
            w13sb1 = wp.tile([P, 2, KH, P], BF16, tag="w13")
            nc.gpsimd.dma_start(w13sb1[:], w13t[1])

            def xh(kh):
                return xa[:, kh, :] if kh < KH2 else xb[:, kh - KH2, :]

            # PE warmup: ramp the tensor engine to high-activity clock while
            # the input DMAs are in flight. Reads a zeroed tile, result is
            # never consumed.
            wtile = warm.tile([P, W_COLS], BF16)
            nc.vector.memset(wtile[:], 0.0)
            # Shares the Phase B psum pool (tag "py"): warmup is long done
            # before Phase B allocates its first chain psum.
            wps = psB.tile([P, W_COLS], F32, tag="py")
            for i in range(N_WARM):
                nc.tensor.matmul(
                    wps, wtile[:, :P], wtile[:], start=(i == 0),
                    stop=(i == N_WARM - 1),
                )

            psb = pp.tile([P, II, C], BF16)

            # Phase A: h1 = silu(x@w1), h3 = x@w3, p = h1*h3 (all transposed)
            for ii in range(II):
                if ii == 0:
                    w13sb = w13sb0
                elif ii == 1:
                    w13sb = w13sb1
                else:
                    w13sb = wp.tile([P, 2, KH, P], BF16, tag="w13")
                    # Alternate weight-stream tiles between the gpsimd and
                    # scalar rings so each only needs ~75GB/s sustained.
                    eng = nc.gpsimd if ii % 2 == 0 else nc.scalar
                    eng.dma_start(w13sb[:], w13t[ii])
                pg = psA.tile([P, C], F32, tag="pg")
                pu = psA.tile([P, C], F32, tag="pu")
                for kh in range(KH):
                    nc.tensor.matmul(
                        pg,
                        w13sb[:, 0, kh, :],
                        xh(kh),
                        start=(kh == 0),
                        stop=(kh == KH - 1),
                    )
                for kh in range(KH):
                    nc.tensor.matmul(
                        pu,
                        w13sb[:, 1, kh, :],
                        xh(kh),
                        start=(kh == 0),
                        stop=(kh == KH - 1),
                    )
                gs = gp.tile([P, C], BF16, tag="g")
                nc.scalar.activation(gs, pg, mybir.ActivationFunctionType.Silu)
                nc.vector.tensor_tensor(
                    psb[:, ii, :], gs, pu, mybir.AluOpType.mult
                )

            # Phase B: y = p @ w2 (transposed: yT = w2T-contraction over I).
            # The last hh is split column-wise so its first half's copy+DMA
            # overlaps the second half's matmuls (shorter kernel tail).
            for hh in range(KH):
                w2sb = w2p.tile([P, II, P], BF16, tag="w2")
                eng = nc.scalar if hh % 2 == 0 else nc.gpsimd
                eng.dma_start(w2sb[:], w2t[hh])
                halves = [(0, C)] if hh < KH - 1 else [
                    (0, C // 2), (C // 2, C - C // 2),
                ]
                for hi, (c0, cc) in enumerate(halves):
                    py = psB.tile([P, cc], F32, tag="py")
                    for ik in range(II):
                        nc.tensor.matmul(
                            py,
                            w2sb[:, ik, :],
                            psb[:, ik, c0 : c0 + cc],
                            start=(ik == 0),
                            stop=(ik == II - 1),
                        )
                    ysb = yp.tile([P, cc], F32, tag="y")
                    nc.scalar.copy(ysb, py)
                    if hh < KH - 1 or hi == 0:
                        nc.sync.dma_start(yt[hh, :, c0 : c0 + cc], ysb[:])
                    else:
                        # Final chunk: split the store across both hardware
                        # DGE rings to shorten the kernel tail.
                        h2 = cc // 2
                        nc.sync.dma_start(
                            yt[hh, :, c0 : c0 + h2], ysb[:, :h2]
                        )
                        nc.scalar.dma_start(
                            yt[hh, :, c0 + h2 : c0 + cc], ysb[:, h2:]
                        )

    nc.compile()
    return nc


_PROGRAM_CACHE = {}


def _host_swiglu(x, w1e, w2e, w3e):
    g = x @ w1e
    u = x @ w3e
    g = g / (1.0 + np.exp(-g))
    return (g * u) @ w2e


def kernel(x, expert_indices, expert_weights, w1, w2, w3):
    x = np.asarray(x, dtype=np.float32)
    idx = np.asarray(expert_indices)
    wts = np.asarray(expert_weights, dtype=np.float32)
    w1 = np.asarray(w1, dtype=np.float32)
    w2 = np.asarray(w2, dtype=np.float32)
    w3 = np.asarray(w3, dtype=np.float32)
    N = x.shape[0]
    E = w1.shape[0]
    bf16 = ml_dtypes.bfloat16

    # host-side routing: token list (with multiplicity) per expert; tokens
    # beyond CAP spill to the host f32 path (tiny tail, keeps device at one
    # full-width PE chunk)
    toks, tokw, spill_toks, spill_w = [], [], [], []
    for e in range(E):
        rows, cols = np.nonzero(idx == e)
        w_e = wts[rows, cols]
        toks.append(rows[:CAP])
        tokw.append(w_e[:CAP])
        spill_toks.append(rows[CAP:])
        spill_w.append(w_e[CAP:])
    C = max(16, max(len(t) for t in toks))
    C = ((C + 15) // 16) * 16

    if C not in _PROGRAM_CACHE:
        _PROGRAM_CACHE[C] = _build(C)
    nc = _PROGRAM_CACHE[C]

    in_maps = []
    for e in range(E):
        xt = np.zeros((C, H), dtype=np.float32)
        if len(toks[e]):
            xt[: len(toks[e])] = x[toks[e]]
        # [C, H] -> [hp, kh, c]
        xge = xt.T.reshape(KH, P, C).transpose(1, 0, 2)
        # w1/w3 [H, I] -> [ii, hp, {w1,w3}, kh, m]
        w13 = np.stack(
            [
                w1[e].reshape(KH, P, II, P).transpose(2, 1, 0, 3),
                w3[e].reshape(KH, P, II, P).transpose(2, 1, 0, 3),
            ],
            axis=2,
        )  # [II, P, 2, KH, P]
        in_maps.append(
            {
                "xg": np.ascontiguousarray(xge.astype(bf16)),
                "w13t": np.ascontiguousarray(w13.astype(bf16)),
                "w2t": np.ascontiguousarray(
                    w2[e].reshape(II, P, KH, P).transpose(2, 1, 0, 3).astype(bf16)
                ),
            }
        )

    res = run_bass_kernel_spmd(nc, in_maps, core_ids=list(range(E)))

    out = np.zeros((N, H), dtype=np.float32)
    for e in range(E):
        cnt = len(toks[e])
        if cnt:
            y = res.results[e]["yt"].reshape(H, C).T[:cnt]
            np.add.at(out, toks[e], y * tokw[e][:, None])
        if len(spill_toks[e]):
            ys = _host_swiglu(x[spill_toks[e]], w1[e], w2[e], w3[e])
            np.add.at(out, spill_toks[e], ys * spill_w[e][:, None])
    return out


# revision 11
# speedup vs baseline: 1.1514x; 1.0002x over previous
"""Expert-parallel MoE SwiGLU kernel for one TRN2 chip (8 NeuronCores).

Problem: out[n] = sum_k w[n,k] * FFN_{idx[n,k]}(x[n]) with E=8 experts,
top-2 routing, H=1024, I=4096, N=2048 tokens.

Strategy: one expert per core. Tokens are routed (gathered) per expert on
the host, each core runs the three bf16 matmuls of its expert's SwiGLU FFN
(silu(x@w1) * (x@w3)) @ w2 over its token batch entirely transposed
(tokens along the PE moving/free dim), and the host scatter-adds the
returned per-expert outputs with the routing weights. Expert token counts
above the per-core capacity (PE moving-dim chunk of 512) spill to a small
host-side f32 pass so the device runs a single full-width chunk.

Schedule notes (from NTFF profile analysis): the kernel is tensor-bound
(768 N=512 bf16 matmuls ~= 166us at 2.4GHz). The startup-critical inputs
(x 1MB + first w13 tile 0.5MB) are split across the three independent DMA
rings (sync + scalar hardware DGE, gpsimd software DGE, ~150GB/s each) so
the first real matmul issues ~11us after kernel start instead of ~21us.
The w13/w2 weight streams alternate between the gpsimd and scalar rings so
each ring only needs ~75GB/s sustained. PE warmup matmuls (HAM clock ramp)
cover the input DMA window.
"""

import sys

for _p in ("/opt/trn_rl_repo", "/opt/pypackages"):
    if _p not in sys.path:
        sys.path.insert(0, _p)

import numpy as np
import ml_dtypes

import concourse.tile as tile
from concourse import bacc, mybir
from concourse.bass_utils import run_bass_kernel_spmd

P = 128
H = 1024
I = 4096
KH = H // P    # 8 contraction subtiles for the first matmuls
KH2 = KH // 2  # kh subtiles per x half-tile
II = I // P    # 32 intermediate subtiles / contraction subtiles for w2
CAP = 512      # per-core token capacity (single PE moving chunk)
N_WARM = 27    # PE warmup matmuls (HAM clock ramp) during the input DMA wait
W_COLS = 256   # warmup matmul moving width (finer tail granularity)

BF16 = mybir.dt.bfloat16
F32 = mybir.dt.float32


def _build(C):
    """One-expert SwiGLU FFN over C tokens (C <= 512), transposed layout.

    DRAM inputs (per core):
      xg   [P, KH, C]       bf16  x^T: [hp, kh, c] = x[tok c, kh*P+hp]
      w13t [II, P, 2, KH, P] bf16 [ii, hp, 0, kh, m] = w1[kh*P+hp, ii*P+m]
                                  [ii, hp, 1, kh, m] = w3[kh*P+hp, ii*P+m]
      w2t  [KH, P, II, P]   bf16  [hh, ip, ik, m] = w2[ik*P+ip, hh*P+m]
    Output:
      yt   [KH, P, C]       f32   y^T tiled by output subtile
    """
    assert C <= 512
    nc = bacc.Bacc("TRN2", target_bir_lowering=False, debug=False)
    xg = nc.dram_tensor("xg", [P, KH, C], BF16, kind="ExternalInput")
    w13t = nc.dram_tensor("w13t", [II, P, 2, KH, P], BF16, kind="ExternalInput")
    w2t = nc.dram_tensor("w2t", [KH, P, II, P], BF16, kind="ExternalInput")
    yt = nc.dram_tensor("yt", [KH, P, C], F32, kind="ExternalOutput")

    with tile.TileContext(nc) as tc:
        with (
            tc.tile_pool(name="xp", bufs=1) as xp,
            tc.tile_pool(name="pp", bufs=1) as pp,
            tc.tile_pool(name="wp", bufs=10) as wp,
            tc.tile_pool(name="w2p", bufs=3) as w2p,
            tc.tile_pool(name="gp", bufs=4) as gp,
            tc.tile_pool(name="yp", bufs=3) as yp,
            tc.tile_pool(name="warm", bufs=1) as warm,
            tc.tile_pool(name="psA", bufs=3, space="PSUM") as psA,
            tc.tile_pool(name="psB", bufs=2, space="PSUM") as psB,
        ):
            # Startup-critical loads, spread over the independent DMA rings
            # (~150GB/s each) in the order the rings come up: sync ~8.8us,
            # scalar ~10.2us (after its ACT table load), gpsimd software DGE
            # ~12.1us (ucode desc-gen latency). Chunked DMAs + subtile deps
            # let the first matmul chains track arriving data.
            xsb = xp.tile([P, KH, C], BF16)
            w13sb0 = wp.tile([P, 2, KH, P], BF16, tag="w13")
            nc.scalar.dma_start(w13sb0[:, 0], w13t[0][:, 0])   # w1 half
            nc.sync.dma_start(xsb[:, 0:2, :], xg[:, 0:2, :])
            nc.sync.dma_start(xsb[:, 2:4, :], xg[:, 2:4, :])
            nc.scalar.dma_start(xsb[:, 4:6, :], xg[:, 4:6, :])
            nc.scalar.dma_start(xsb[:, 6:8, :], xg[:, 6:8, :])
            nc.sync.dma_start(w13sb0[:, 1], w13t[0][:, 1])     # w3 half
            w13sb1 = wp.tile([P, 2, KH, P], BF16, tag="w13")
            nc.gpsimd.dma_start(w13sb1[:], w13t[1])

            def xh(kh):
                return xsb[:, kh, :]

            # PE warmup: ramp the tensor engine to high-activity clock while
            # the input DMAs are in flight. Reads a zeroed tile, result is
            # never consumed.
            wtile = warm.tile([P, W_COLS], BF16)
            nc.vector.memset(wtile[:], 0.0)
            # Shares the Phase B psum pool (tag "py"): warmup is long done
            # before Phase B allocates its first chain psum.
            wps = psB.tile([P, W_COLS], F32, tag="py")
            for i in range(N_WARM):
                nc.tensor.matmul(
                    wps, wtile[:, :P], wtile[:], start=(i == 0),
                    stop=(i == N_WARM - 1),
                )

            psb = pp.tile([P, II, C], BF16)

            # Phase A: h1 = silu(x@w1), h3 = x@w3, p = h1*h3 (all transposed)
            for ii in range(II):
                if ii == 0:
                    w13sb = w13sb0
                elif ii == 1:
                    w13sb = w13sb1
                else:
                    w13sb = wp.tile([P, 2, KH, P], BF16, tag="w13")
                    # Alternate weight-stream tiles between the gpsimd and
                    # scalar rings so each only needs ~75GB/s sustained.
                    # ii=2,3 both ride gpsimd: scalar is still busy with the
                    # startup-critical x chunks at that point.
                    if ii in (2, 3):
                        eng = nc.gpsimd
                    else:
                        eng = nc.gpsimd if ii % 2 == 0 else nc.scalar
                    eng.dma_start(w13sb[:], w13t[ii])
                pg = psA.tile([P, C], F32, tag="pg")
                pu = psA.tile([P, C], F32, tag="pu")
                for kh in range(KH):
                    nc.tensor.matmul(
                        pg,
                        w13sb[:, 0, kh, :],
                        xh(kh),
                        start=(kh == 0),
                        stop=(kh == KH - 1),
                    )
                for kh in range(KH):
                    nc.tensor.matmul(
                        pu,
                        w13sb[:, 1, kh, :],
                        xh(kh),
                        start=(kh == 0),
                        stop=(kh == KH - 1),
                    )
                gs = gp.tile([P, C], BF16, tag="g")
                nc.scalar.activation(gs, pg, mybir.ActivationFunctionType.Silu)
                nc.vector.tensor_tensor(
                    psb[:, ii, :], gs, pu, mybir.AluOpType.mult
                )

            # Phase B: y = p @ w2 (transposed: yT = w2T-contraction over I).
            # The last hh is split column-wise so its first half's copy+DMA
            # overlaps the second half's matmuls (shorter kernel tail).
            for hh in range(KH):
                w2sb = w2p.tile([P, II, P], BF16, tag="w2")
                # hh=0,1 on scalar (its w13 stream drains first at the A->B
                # transition; the gpsimd ring is still backlogged), then
                # alternate.
                eng = nc.scalar if (hh < 2 or hh % 2 == 1) else nc.gpsimd
                eng.dma_start(w2sb[:], w2t[hh])
                halves = [(0, C)] if hh < KH - 1 else [
                    (0, C // 2), (C // 2, C - C // 2),
                ]
                for hi, (c0, cc) in enumerate(halves):
                    py = psB.tile([P, cc], F32, tag="py")
                    for ik in range(II):
                        nc.tensor.matmul(
                            py,
                            w2sb[:, ik, :],
                            psb[:, ik, c0 : c0 + cc],
                            start=(ik == 0),
                            stop=(ik == II - 1),
                        )
                    ysb = yp.tile([P, cc], F32, tag="y")
                    # DVE copy keeps the COPY activation table off the scalar
                    # queue (its ACT_TABLE_LOAD would delay the scalar DMA
                    # ring's startup-critical triggers by ~1.3us).
                    nc.vector.tensor_copy(ysb, py)
                    if hh < KH - 1 or hi == 0:
                        nc.sync.dma_start(yt[hh, :, c0 : c0 + cc], ysb[:])
                    else:
                        # Final chunk: split the store across both hardware
                        # DGE rings to shorten the kernel tail.
                        h2 = cc // 2
                        nc.sync.dma_start(
                            yt[hh, :, c0 : c0 + h2], ysb[:, :h2]
                        )
                        nc.scalar.dma_start(
                            yt[hh, :, c0 + h2 : c0 + cc], ysb[:, h2:]
                        )

    nc.compile()
    return nc


_PROGRAM_CACHE = {}


def _host_swiglu(x, w1e, w2e, w3e):
    g = x @ w1e
    u = x @ w3e
    g = g / (1.0 + np.exp(-g))
    return (g * u) @ w2e


def kernel(x, expert_indices, expert_weights, w1, w2, w3):
    x = np.asarray(x, dtype=np.float32)
    idx = np.asarray(expert_indices)
    wts = np.asarray(expert_weights, dtype=np.float32)
    w1 = np.asarray(w1, dtype=np.float32)
    w2 = np.asarray(w2, dtype=np.float32)
    w3 = np.asarray(w3, dtype=np.float32)
    N = x.shape[0]
    E = w1.shape[0]
    bf16 = ml_dtypes.bfloat16

    # host-side routing: token list (with multiplicity) per expert; tokens
    # beyond CAP spill to the host f32 path (tiny tail, keeps device at one
    # full-width PE chunk)
    toks, tokw, spill_toks, spill_w = [], [], [], []
    for e in range(E):
        rows, cols = np.nonzero(idx == e)
        w_e = wts[rows, cols]
        toks.append(rows[:CAP])
        tokw.append(w_e[:CAP])
        spill_toks.append(rows[CAP:])
        spill_w.append(w_e[CAP:])
    C = max(16, max(len(t) for t in toks))
    C = ((C + 15) // 16) * 16

    if C not in _PROGRAM_CACHE:
        _PROGRAM_CACHE[C] = _build(C)
    nc = _PROGRAM_CACHE[C]

    in_maps = []
    for e in range(E):
        xt = np.zeros((C, H), dtype=np.float32)
        if len(toks[e]):
            xt[: len(toks[e])] = x[toks[e]]
        # [C, H] -> [hp, kh, c]
        xge = xt.T.reshape(KH, P, C).transpose(1, 0, 2)
        # w1/w3 [H, I] -> [ii, hp, {w1,w3}, kh, m]
        w13 = np.stack(
            [
                w1[e].reshape(KH, P, II, P).transpose(2, 1, 0, 3),
                w3[e].reshape(KH, P, II, P).transpose(2, 1, 0, 3),
            ],
            axis=2,
        )  # [II, P, 2, KH, P]
        in_maps.append(
            {
                "xg": np.ascontiguousarray(xge.astype(bf16)),
                "w13t": np.ascontiguousarray(w13.astype(bf16)),
                "w2t": np.ascontiguousarray(
                    w2[e].reshape(II, P, KH, P).transpose(2, 1, 0, 3).astype(bf16)
                ),
            }
        )

    res = run_bass_kernel_spmd(nc, in_maps, core_ids=list(range(E)))

    out = np.zeros((N, H), dtype=np.float32)
    for e in range(E):
        cnt = len(toks[e])
        if cnt:
            y = res.results[e]["yt"].reshape(H, C).T[:cnt]
            np.add.at(out, toks[e], y * tokw[e][:, None])
        if len(spill_toks[e]):
            ys = _host_swiglu(x[spill_toks[e]], w1[e], w2[e], w3[e])
            np.add.at(out, spill_toks[e], ys * spill_w[e][:, None])
    return out
